# revision 5
# baseline (speedup 1.0000x reference)
"""Trainium2 Bass kernel for nn_MHABlock (dense transformer block).

Sharding: data-parallel over batch — 8 cores x 4 batches (2048 tokens/core).
BatchNorm stats are exact via two tiny cross-core AllReduces ([128,2] each).

On-device layout is E-major ("hT" = [E(128 partitions), tokens(free)]) so
BatchNorm / bias / affine ops are per-partition scalars. Attention uses the
"scoresT" formulation (scores transposed: [k_tok, q_tok]): softmax sums
arrive free from an all-ones column prepended to V (row 32*hh of the attnV
PSUM = sum_k exp), no transpose of the attention matrix is needed, and the
exp output feeds attnV directly as the streaming operand. Per-head softmax
denominators land on partitions {0,32,64,96}; reciprocal_approx_fast over
the whole bank + K=1 outer-product matmuls with a ones vector broadcast
1/sum across each head's 32-partition group, so normalization is a single
[128,512] tensor_tensor multiply. Zero rows in the out-projection weights
kill all pad rows. q/k/exp/V run in bf16 (PE fp32 is half-rate).
"""

import numpy as np

B, N, D_IN, E, H, KD, FF = 32, 512, 2, 128, 8, 16, 512
NCORES = 8
BPC = B // NCORES          # batches per core
T = BPC * N                # 2048 local tokens
NTOK = B * N               # global token count for BN
NORM = 1.0 / np.sqrt(16.0)
EPS = 1e-5

_CACHE = {}
LAST_RESULT = None


def _build_nc():
    import concourse.bass as bass  # noqa: F401
    import concourse.mybir as mybir
    import concourse.tile as tile
    from concourse import bacc

    f32 = mybir.dt.float32
    bf16 = mybir.dt.bfloat16
    Act = mybir.ActivationFunctionType
    Alu = mybir.AluOpType
    AX = mybir.AxisListType

    nc = bacc.Bacc("TRN2", target_bir_lowering=False, debug=False,
                   enable_asserts=False, num_devices=NCORES)

    # ---- DRAM I/O ----
    d_xT = nc.dram_tensor("xT", [D_IN, T], f32, kind="ExternalInput").ap()
    d_We1 = nc.dram_tensor("We1", [D_IN, E], f32, kind="ExternalInput").ap()
    d_WqQ = nc.dram_tensor("WqQ", [E, 256], f32, kind="ExternalInput").ap()
    d_WkQ = nc.dram_tensor("WkQ", [E, 256], f32, kind="ExternalInput").ap()
    d_WvI = nc.dram_tensor("WvI", [E, 128], f32, kind="ExternalInput").ap()
    d_WoQ = nc.dram_tensor("WoQ", [128, 256], f32, kind="ExternalInput").ap()
    d_fW1 = nc.dram_tensor("fW1", [E, FF], f32, kind="ExternalInput").ap()
    d_fW2 = nc.dram_tensor("fW2", [128, 512], f32, kind="ExternalInput").ap()
    d_vecs = nc.dram_tensor("vecs", [128, 12], f32, kind="ExternalInput").ap()
    d_yT = nc.dram_tensor("yT", [E, T], f32, kind="ExternalOutput").ap()

    RG = [list(range(NCORES))]

    with tile.TileContext(nc) as tc:
        with tc.sbuf_pool(name="sb", bufs=1) as sb, \
             tc.psum_pool(name="ps", bufs=1) as ps, \
             tc.tile_pool(name="dr", bufs=1, space="DRAM") as dr:

            def P(shape, dt, name):  # persistent tile
                return sb.tile(shape, dt, name=name, tag=name, bufs=1)

            xT = P([D_IN, T], f32, "xT_sb")
            We1_sb = P([D_IN, E], f32, "We1_sb")
            WqQ_sb = P([128, 256], f32, "WqQ_sb")
            WkQ_sb = P([128, 256], f32, "WkQ_sb")
            WvI_sb = P([128, 128], f32, "WvI_sb")
            WoQ_sb = P([128, 256], f32, "WoQ_sb")
            fW1_sb = P([128, FF], f32, "fW1_sb")
            fW2_sb = P([128, 512], f32, "fW2_sb")
            vecs_sb = P([128, 12], f32, "vecs_sb")
            ones_sb = P([128, 32], f32, "ones_sb")

            H0T = P([128, T], f32, "H0T")
            qT = [P([128, T], bf16, f"qT{g}") for g in range(2)]
            kT = [P([128, T], bf16, f"kT{g}") for g in range(2)]
            V_aug = P([128, 16 * 256], bf16, "V_aug")
            HT = [P([128, T], f32, f"HT{g}") for g in range(2)]
            h1T = P([128, T], f32, "h1T")
            h1nT = P([128, T], f32, "h1nT")
            h2T = [P([128, T], f32, f"h2T{qf}") for qf in range(4)]
            yT = P([128, T], f32, "yT_sb")
            sq = P([128, T], f32, "sq")
            st1 = P([128, 2], f32, "st1")
            st2 = P([128, 2], f32, "st2")
            gst1 = P([128, 2], f32, "gst1")
            gst2 = P([128, 2], f32, "gst2")
            bn1s = P([128, 6], f32, "bn1s")
            bn2s = P([128, 6], f32, "bn2s")

            # ---- load inputs ----
            nc.sync.dma_start(xT[:], d_xT)
            nc.sync.dma_start(We1_sb[:], d_We1)
            nc.sync.dma_start(WqQ_sb[:], d_WqQ)
            nc.sync.dma_start(WkQ_sb[:], d_WkQ)
            nc.sync.dma_start(WvI_sb[:], d_WvI)
            nc.sync.dma_start(WoQ_sb[:], d_WoQ)
            nc.sync.dma_start(fW1_sb[:], d_fW1)
            nc.sync.dma_start(fW2_sb[:], d_fW2)
            nc.sync.dma_start(vecs_sb[:], d_vecs)
            nc.vector.memset(ones_sb[:], 1.0)
            nc.vector.memset(V_aug[:], 0.0)
            va_ones = V_aug.rearrange("p (t h w) -> p (t h) w", t=16, h=8)[:, :, 0:1]
            nc.gpsimd.memset(va_ones, 1.0)

            # ---- Phase A: embedding h0 = x @ We1 + be1 (E-major) ----
            for c in range(4):
                pm = ps.tile([128, 512], f32, tag="mm", bufs=2, name=f"pm_e{c}")
                nc.tensor.matmul(pm[:], lhsT=We1_sb[:],
                                 rhs=xT[:, 512 * c:512 * (c + 1)],
                                 start=True, stop=True)
                nc.vector.tensor_scalar_add(H0T[:, 512 * c:512 * (c + 1)],
                                            pm[:], vecs_sb[:, 0:1])

            # ---- Phase B: q/k projections (quad-padded, bf16 out) ----
            for g in range(2):
                for c in range(4):
                    pq = ps.tile([128, 512], f32, tag="mm", bufs=2,
                                 name=f"pq{g}{c}")
                    nc.tensor.matmul(pq[:], lhsT=WqQ_sb[:, 128 * g:128 * (g + 1)],
                                     rhs=H0T[:, 512 * c:512 * (c + 1)],
                                     start=True, stop=True)
                    nc.vector.tensor_copy(qT[g][:, 512 * c:512 * (c + 1)], pq[:])
                    pk = ps.tile([128, 512], f32, tag="mm", bufs=2,
                                 name=f"pk{g}{c}")
                    nc.tensor.matmul(pk[:], lhsT=WkQ_sb[:, 128 * g:128 * (g + 1)],
                                     rhs=H0T[:, 512 * c:512 * (c + 1)],
                                     start=True, stop=True)
                    nc.vector.tensor_copy(kT[g][:, 512 * c:512 * (c + 1)], pk[:])

            # ---- v projection into V_aug (token-major, 32-blocks +ones) ----
            for t in range(16):
                pv = ps.tile([128, 128], f32, tag="mm", bufs=2, name=f"pv{t}")
                nc.tensor.matmul(pv[:], lhsT=H0T[:, 128 * t:128 * (t + 1)],
                                 rhs=WvI_sb[:], start=True, stop=True)
                dst = V_aug[:, 256 * t:256 * (t + 1)]
                dst = dst.rearrange("p (h w) -> p h w", h=8)[:, :, 1:17]
                src = pv.rearrange("p (h w) -> p h w", h=8)
                nc.vector.tensor_copy(dst, src)

            # ---- Phase C: attention ----
            for b in range(4):
                for g in range(2):
                    av = ps.tile([128, 512], f32, tag="av", bufs=2,
                                 name=f"av{b}{g}")
                    for c in range(4):
                        for hp in range(2):
                            scp = ps.tile([128, 1024], f32, tag="sc", bufs=2,
                                          name=f"scp{b}{g}{c}{hp}")
                            for j in range(2):
                                hh = 2 * hp + j
                                nc.tensor.matmul(
                                    scp[:, 512 * j:512 * (j + 1)],
                                    lhsT=kT[g][32 * hh:32 * (hh + 1),
                                               512 * b + 128 * c:
                                               512 * b + 128 * (c + 1)],
                                    rhs=qT[g][32 * hh:32 * (hh + 1),
                                              512 * b:512 * (b + 1)],
                                    start=True, stop=True,
                                    tile_position=(32 * hh, 0))
                            ex = sb.tile([128, 1024], bf16, tag="ex", bufs=3,
                                         name=f"ex{b}{g}{c}{hp}")
                            nc.scalar.activation(ex[:], scp[:], Act.Exp,
                                                 scale=float(NORM))
                            for j in range(2):
                                hh = 2 * hp + j
                                h = 4 * g + hh
                                tci = 4 * b + c
                                nc.tensor.matmul(
                                    av[32 * hh:32 * (hh + 1), :],
                                    lhsT=V_aug[:, 256 * tci + 32 * h:
                                               256 * tci + 32 * h + 32],
                                    rhs=ex[:, 512 * j:512 * (j + 1)],
                                    start=(c == 0), stop=(c == 3),
                                    tile_position=(0, 32 * hh))
                    # normalize: raw * (1/sums), broadcast via K=1 matmul
                    raw = sb.tile([128, 512], f32, tag="raw", bufs=2,
                                  name=f"raw{b}{g}")
                    nc.vector.tensor_copy(raw[:], av[:])
                    rec = sb.tile([128, 512], f32, tag="rec", bufs=2,
                                  name=f"rec{b}{g}")
                    nc.vector.reciprocal_approx_fast(rec[:], av[:])
                    Rp = ps.tile([128, 512], f32, tag="mm", bufs=2,
                                 name=f"Rp{b}{g}")
                    for hh in range(4):
                        nc.tensor.matmul(
                            Rp[32 * hh:32 * (hh + 1), :],
                            lhsT=ones_sb[32 * hh:32 * hh + 1, :],
                            rhs=rec[32 * hh:32 * hh + 1, :],
                            start=True, stop=True,
                            tile_position=(32 * hh, 32 * hh))
                    nc.vector.tensor_mul(HT[g][:, 512 * b:512 * (b + 1)],
                                         raw[:], Rp[:])

                # out-projection + skip (both quads accumulate in PSUM)
                po = ps.tile([128, 512], f32, tag="mm", bufs=2, name=f"po{b}")
                for g in range(2):
                    nc.tensor.matmul(po[:], lhsT=WoQ_sb[:, 128 * g:128 * (g + 1)],
                                     rhs=HT[g][:, 512 * b:512 * (b + 1)],
                                     start=(g == 0), stop=(g == 1))
                nc.vector.tensor_add(h1T[:, 512 * b:512 * (b + 1)], po[:],
                                     H0T[:, 512 * b:512 * (b + 1)])

            # ---- BatchNorm helper (exact, cross-core stats) ----
            def batchnorm(src, st, gst, bns, wcol, bcol, ccname):
                nc.vector.reduce_sum(out=st[:, 0:1], in_=src[:], axis=AX.X)
                nc.scalar.activation(sq[:], src[:], Act.Square,
                                     accum_out=st[:, 1:2])
                cc_in = dr.tile([128, 2], f32, name=f"{ccname}_in",
                                tag=f"{ccname}_in")
                cc_out = dr.tile([128, 2], f32, addr_space="Shared",
                                 name=f"{ccname}_out", tag=f"{ccname}_out")
                nc.sync.dma_start(cc_in[:], st[:])
                nc.gpsimd.collective_compute(
                    "AllReduce", Alu.add, replica_groups=RG,
                    ins=[cc_in[:]], outs=[cc_out[:]])
                nc.sync.dma_start(gst[:], cc_out[:])
                inv_n = 1.0 / float(NTOK)
                nc.vector.tensor_scalar_mul(bns[:, 0:1], gst[:, 0:1], inv_n)
                nc.vector.tensor_scalar_mul(bns[:, 1:2], gst[:, 1:2], inv_n)
                nc.vector.tensor_mul(bns[:, 4:5], bns[:, 0:1], bns[:, 0:1])
                nc.vector.tensor_sub(bns[:, 1:2], bns[:, 1:2], bns[:, 4:5])
                nc.scalar.activation(bns[:, 5:6], bns[:, 1:2], Act.Sqrt,
                                     bias=vecs_sb[:, 9:10])
                nc.vector.reciprocal(bns[:, 2:3], bns[:, 5:6])
                nc.vector.tensor_mul(bns[:, 2:3], bns[:, 2:3],
                                     vecs_sb[:, wcol:wcol + 1])
                nc.vector.tensor_mul(bns[:, 4:5], bns[:, 0:1], bns[:, 2:3])
                nc.vector.tensor_sub(bns[:, 3:4], vecs_sb[:, bcol:bcol + 1],
                                     bns[:, 4:5])

            # ---- BN1 ----
            batchnorm(h1T, st1, gst1, bn1s, 1, 2, "cc1")
            for c in range(4):
                nc.vector.tensor_scalar(
                    h1nT[:, 512 * c:512 * (c + 1)],
                    h1T[:, 512 * c:512 * (c + 1)],
                    bn1s[:, 2:3], bn1s[:, 3:4], op0=Alu.mult, op1=Alu.add)

            # ---- FFN (ffb2 cancels inside BN2) ----
            for qf in range(4):
                for c in range(4):
                    pf = ps.tile([128, 512], f32, tag="mm", bufs=2,
                                 name=f"pf{qf}{c}")
                    nc.tensor.matmul(pf[:],
                                     lhsT=fW1_sb[:, 128 * qf:128 * (qf + 1)],
                                     rhs=h1nT[:, 512 * c:512 * (c + 1)],
                                     start=True, stop=True)
                    nc.vector.tensor_scalar(
                        h2T[qf][:, 512 * c:512 * (c + 1)], pf[:],
                        vecs_sb[:, 3 + qf:4 + qf], 0.0,
                        op0=Alu.add, op1=Alu.max)
            for c in range(4):
                p2 = ps.tile([128, 512], f32, tag="mm", bufs=2, name=f"p2{c}")
                for qf in range(4):
                    nc.tensor.matmul(p2[:],
                                     lhsT=fW2_sb[:, 128 * qf:128 * (qf + 1)],
                                     rhs=h2T[qf][:, 512 * c:512 * (c + 1)],
                                     start=(qf == 0), stop=(qf == 3))
                nc.vector.tensor_add(yT[:, 512 * c:512 * (c + 1)], p2[:],
                                     h1nT[:, 512 * c:512 * (c + 1)])

            # ---- BN2 + output ----
            batchnorm(yT, st2, gst2, bn2s, 7, 8, "cc2")
            for c in range(4):
                nc.vector.tensor_scalar(
                    sq[:, 512 * c:512 * (c + 1)], yT[:, 512 * c:512 * (c + 1)],
                    bn2s[:, 2:3], bn2s[:, 3:4], op0=Alu.mult, op1=Alu.add)
                nc.sync.dma_start(d_yT[:, 512 * c:512 * (c + 1)],
                                  sq[:, 512 * c:512 * (c + 1)])

    nc.compile()
    return nc


def _host_prep(inputs):
    f = np.float32
    Wq, Wk, Wv, Wo = (np.asarray(inputs[k], f) for k in ("Wq", "Wk", "Wv", "Wo"))
    WqQ = np.zeros((2, E, 128), f)
    WkQ = np.zeros((2, E, 128), f)
    WoQ = np.zeros((2, 128, E), f)
    for g in range(2):
        for hh in range(4):
            h = 4 * g + hh
            WqQ[g, :, 32 * hh:32 * hh + 16] = Wq[h]
            WkQ[g, :, 32 * hh:32 * hh + 16] = Wk[h]
            WoQ[g, 32 * hh + 1:32 * hh + 17, :] = Wo[h]
    WvI = np.ascontiguousarray(np.transpose(Wv, (1, 0, 2)).reshape(E, H * KD))
    fW2 = np.ascontiguousarray(
        np.asarray(inputs["ffW2"], f).reshape(4, 128, E).transpose(1, 0, 2))
    vecs = np.zeros((128, 12), f)
    vecs[:, 0] = inputs["be1"]
    vecs[:, 1] = inputs["bn1_w"]
    vecs[:, 2] = inputs["bn1_b"]
    vecs[:, 3:7] = np.asarray(inputs["ffb1"], f).reshape(4, 128).T
    vecs[:, 7] = inputs["bn2_w"]
    vecs[:, 8] = inputs["bn2_b"]
    vecs[:, 9] = EPS
    return {
        "We1": np.ascontiguousarray(np.asarray(inputs["We1"], f)),
        "WqQ": np.ascontiguousarray(np.concatenate([WqQ[0], WqQ[1]], axis=1)),
        "WkQ": np.ascontiguousarray(np.concatenate([WkQ[0], WkQ[1]], axis=1)),
        "WvI": WvI,
        "WoQ": np.ascontiguousarray(np.concatenate([WoQ[0], WoQ[1]], axis=1)),
        "fW1": np.ascontiguousarray(np.asarray(inputs["ffW1"], f)),
        "fW2": np.ascontiguousarray(fW2.reshape(128, 512)), "vecs": vecs,
    }


def kernel(**inputs):
    global LAST_RESULT
    import os
    from concourse.bass_utils import run_bass_kernel_spmd

    if "nc" not in _CACHE:
        _CACHE["nc"] = _build_nc()
    nc = _CACHE["nc"]

    shared = _host_prep(inputs)
    x1 = np.asarray(inputs["x1"], np.float32)
    in_maps = []
    for cidx in range(NCORES):
        m = dict(shared)
        xl = x1[BPC * cidx:BPC * (cidx + 1)].reshape(T, D_IN)
        m["xT"] = np.ascontiguousarray(xl.T)
        in_maps.append(m)

    trace = os.environ.get("KBENCH_TRACE") == "1"
    res = run_bass_kernel_spmd(nc, in_maps, core_ids=list(range(NCORES)),
                               trace=trace)
    LAST_RESULT = res
    outs = []
    for cidx in range(NCORES):
        yTo = res.results[cidx]["yT"]          # [E, T]
        outs.append(np.ascontiguousarray(yTo.T).reshape(BPC, N, E))
    return np.concatenate(outs, 0).astype(np.float32)


# revision 6
# speedup vs baseline: 1.8911x; 1.8911x over previous
"""Trainium2 Bass kernel for nn_MHABlock (dense transformer block).

Sharding: data-parallel over batch — 8 cores x 4 batches (2048 tokens/core).
BatchNorm stats are exact via two tiny cross-core AllReduces ([128,2] each).

On-device layout is E-major ("hT" = [E(128 partitions), tokens(free)]) so
BatchNorm / bias / affine ops are per-partition scalars. Attention uses the
"scoresT" formulation (scores transposed: [k_tok, q_tok]): softmax sums
arrive free from an all-ones column prepended to V (row 32*hh of the attnV
PSUM = sum_k exp), no transpose of the attention matrix is needed, and the
exp output feeds attnV directly as the streaming operand. Per-head softmax
denominators land on partitions {0,32,64,96}; reciprocal_approx_fast over
the whole bank + K=1 outer-product matmuls with a ones vector broadcast
1/sum across each head's 32-partition group, so normalization is a single
[128,512] tensor_tensor multiply. Zero rows in the out-projection weights
kill all pad rows. q/k/exp/V run in bf16 (PE fp32 is half-rate).
"""

import numpy as np

B, N, D_IN, E, H, KD, FF = 32, 512, 2, 128, 8, 16, 512
NCORES = 8
BPC = B // NCORES          # batches per core
T = BPC * N                # 2048 local tokens
NTOK = B * N               # global token count for BN
NORM = 1.0 / np.sqrt(16.0)
EPS = 1e-5

_CACHE = {}
LAST_RESULT = None


def _build_nc():
    import concourse.bass as bass  # noqa: F401
    import concourse.mybir as mybir
    import concourse.tile as tile
    from concourse import bacc

    f32 = mybir.dt.float32
    bf16 = mybir.dt.bfloat16
    Act = mybir.ActivationFunctionType
    Alu = mybir.AluOpType
    AX = mybir.AxisListType

    nc = bacc.Bacc("TRN2", target_bir_lowering=False, debug=False,
                   enable_asserts=False, num_devices=NCORES)

    # ---- DRAM I/O ----
    d_xT = nc.dram_tensor("xT", [D_IN, T], f32, kind="ExternalInput").ap()
    d_We1 = nc.dram_tensor("We1", [D_IN, E], f32, kind="ExternalInput").ap()
    d_WqQ = nc.dram_tensor("WqQ", [E, 256], f32, kind="ExternalInput").ap()
    d_WkQ = nc.dram_tensor("WkQ", [E, 256], f32, kind="ExternalInput").ap()
    d_WvI = nc.dram_tensor("WvI", [E, 128], f32, kind="ExternalInput").ap()
    d_WoQ = nc.dram_tensor("WoQ", [128, 256], f32, kind="ExternalInput").ap()
    d_fW1 = nc.dram_tensor("fW1", [E, FF], f32, kind="ExternalInput").ap()
    d_fW2 = nc.dram_tensor("fW2", [128, 512], f32, kind="ExternalInput").ap()
    d_vecs = nc.dram_tensor("vecs", [128, 12], f32, kind="ExternalInput").ap()
    d_yT = nc.dram_tensor("yT", [E, T], f32, kind="ExternalOutput").ap()

    RG = [list(range(NCORES))]

    with tile.TileContext(nc) as tc:
        with tc.sbuf_pool(name="sb", bufs=1) as sb, \
             tc.psum_pool(name="ps", bufs=1) as ps, \
             tc.tile_pool(name="dr", bufs=1, space="DRAM") as dr:

            def P(shape, dt, name):  # persistent tile
                return sb.tile(shape, dt, name=name, tag=name, bufs=1)

            xT = P([D_IN, T], f32, "xT_sb")
            We1_sb = P([D_IN, E], f32, "We1_sb")
            WqQ_sb = P([128, 256], f32, "WqQ_sb")
            WkQ_sb = P([128, 256], f32, "WkQ_sb")
            WvI_sb = P([128, 128], f32, "WvI_sb")
            WoQ_sb = P([128, 256], f32, "WoQ_sb")
            fW1_sb = P([128, FF], f32, "fW1_sb")
            fW2_sb = P([128, 512], f32, "fW2_sb")
            vecs_sb = P([128, 12], f32, "vecs_sb")
            ones_sb = P([128, 32], f32, "ones_sb")

            H0T = P([128, T], f32, "H0T")
            qT = [P([128, T], bf16, f"qT{g}") for g in range(2)]
            kT = [P([128, T], bf16, f"kT{g}") for g in range(2)]
            V_aug = P([128, 16 * 256], bf16, "V_aug")
            HT = [P([128, T], f32, f"HT{g}") for g in range(2)]
            h1T = P([128, T], f32, "h1T")
            h1nT = P([128, T], f32, "h1nT")
            h2T = [P([128, T], f32, f"h2T{qf}") for qf in range(4)]
            yT = P([128, T], f32, "yT_sb")
            sq = P([128, T], f32, "sq")
            st1 = P([128, 2], f32, "st1")
            st2 = P([128, 2], f32, "st2")
            gst1 = P([128, 2], f32, "gst1")
            gst2 = P([128, 2], f32, "gst2")
            bn1s = P([128, 6], f32, "bn1s")
            bn2s = P([128, 6], f32, "bn2s")

            # ---- load inputs ----
            nc.sync.dma_start(xT[:], d_xT)
            nc.sync.dma_start(We1_sb[:], d_We1)
            nc.sync.dma_start(WqQ_sb[:], d_WqQ)
            nc.sync.dma_start(WkQ_sb[:], d_WkQ)
            nc.sync.dma_start(WvI_sb[:], d_WvI)
            nc.sync.dma_start(WoQ_sb[:], d_WoQ)
            nc.sync.dma_start(fW1_sb[:], d_fW1)
            nc.sync.dma_start(fW2_sb[:], d_fW2)
            nc.sync.dma_start(vecs_sb[:], d_vecs)
            nc.vector.memset(ones_sb[:], 1.0)
            nc.vector.memset(V_aug[:], 0.0)
            va_ones = V_aug.rearrange("p (t h w) -> p (t h) w", t=16, h=8)[:, :, 0:1]
            nc.gpsimd.memset(va_ones, 1.0)

            # ---- Phase A: embedding h0 = x @ We1 + be1 (E-major) ----
            for c in range(4):
                pm = ps.tile([128, 512], f32, tag="mm", bufs=2, name=f"pm_e{c}")
                nc.tensor.matmul(pm[:], lhsT=We1_sb[:],
                                 rhs=xT[:, 512 * c:512 * (c + 1)],
                                 start=True, stop=True)
                nc.vector.tensor_scalar_add(H0T[:, 512 * c:512 * (c + 1)],
                                            pm[:], vecs_sb[:, 0:1])

            # ---- Phase B: q/k projections (quad-padded, bf16 out) ----
            for g in range(2):
                for c in range(4):
                    pq = ps.tile([128, 512], f32, tag="mm", bufs=2,
                                 name=f"pq{g}{c}")
                    nc.tensor.matmul(pq[:], lhsT=WqQ_sb[:, 128 * g:128 * (g + 1)],
                                     rhs=H0T[:, 512 * c:512 * (c + 1)],
                                     start=True, stop=True)
                    nc.vector.tensor_copy(qT[g][:, 512 * c:512 * (c + 1)], pq[:])
                    pk = ps.tile([128, 512], f32, tag="mm", bufs=2,
                                 name=f"pk{g}{c}")
                    nc.tensor.matmul(pk[:], lhsT=WkQ_sb[:, 128 * g:128 * (g + 1)],
                                     rhs=H0T[:, 512 * c:512 * (c + 1)],
                                     start=True, stop=True)
                    nc.vector.tensor_copy(kT[g][:, 512 * c:512 * (c + 1)], pk[:])

            # ---- v projection into V_aug (token-major, 32-blocks +ones) ----
            for t in range(16):
                pv = ps.tile([128, 128], f32, tag="mm", bufs=2, name=f"pv{t}")
                nc.tensor.matmul(pv[:], lhsT=H0T[:, 128 * t:128 * (t + 1)],
                                 rhs=WvI_sb[:], start=True, stop=True)
                dst = V_aug[:, 256 * t:256 * (t + 1)]
                dst = dst.rearrange("p (h w) -> p h w", h=8)[:, :, 1:17]
                src = pv.rearrange("p (h w) -> p h w", h=8)
                nc.vector.tensor_copy(dst, src)

            # ---- Phase C: attention ----
            for b in range(4):
                for g in range(2):
                    av = ps.tile([128, 512], f32, tag="av", bufs=2,
                                 name=f"av{b}{g}")
                    for c in range(4):
                        for hp in range(2):
                            scp = ps.tile([128, 1024], f32, tag="sc", bufs=2,
                                          name=f"scp{b}{g}{c}{hp}")
                            for j in range(2):
                                hh = 2 * hp + j
                                nc.tensor.matmul(
                                    scp[:, 512 * j:512 * (j + 1)],
                                    lhsT=kT[g][32 * hh:32 * (hh + 1),
                                               512 * b + 128 * c:
                                               512 * b + 128 * (c + 1)],
                                    rhs=qT[g][32 * hh:32 * (hh + 1),
                                              512 * b:512 * (b + 1)],
                                    start=True, stop=True,
                                    tile_position=(32 * hh, 0))
                            ex = sb.tile([128, 1024], bf16, tag="ex", bufs=3,
                                         name=f"ex{b}{g}{c}{hp}")
                            nc.scalar.activation(ex[:], scp[:], Act.Exp,
                                                 scale=float(NORM))
                            for j in range(2):
                                hh = 2 * hp + j
                                h = 4 * g + hh
                                tci = 4 * b + c
                                nc.tensor.matmul(
                                    av[32 * hh:32 * (hh + 1), :],
                                    lhsT=V_aug[:, 256 * tci + 32 * h:
                                               256 * tci + 32 * h + 32],
                                    rhs=ex[:, 512 * j:512 * (j + 1)],
                                    start=(c == 0), stop=(c == 3),
                                    tile_position=(0, 32 * hh))
                    # normalize: raw * (1/sums), broadcast via K=1 matmul
                    raw = sb.tile([128, 512], f32, tag="raw", bufs=2,
                                  name=f"raw{b}{g}")
                    nc.vector.tensor_copy(raw[:], av[:])
                    rec = sb.tile([128, 512], f32, tag="rec", bufs=2,
                                  name=f"rec{b}{g}")
                    nc.vector.reciprocal_approx_fast(rec[:], av[:])
                    Rp = ps.tile([128, 512], f32, tag="mm", bufs=2,
                                 name=f"Rp{b}{g}")
                    for hh in range(4):
                        nc.tensor.matmul(
                            Rp[32 * hh:32 * (hh + 1), :],
                            lhsT=ones_sb[32 * hh:32 * hh + 1, :],
                            rhs=rec[32 * hh:32 * hh + 1, :],
                            start=True, stop=True,
                            tile_position=(32 * hh, 32 * hh))
                    nc.vector.tensor_mul(HT[g][:, 512 * b:512 * (b + 1)],
                                         raw[:], Rp[:])

                # out-projection + skip (both quads accumulate in PSUM)
                po = ps.tile([128, 512], f32, tag="mm", bufs=2, name=f"po{b}")
                for g in range(2):
                    nc.tensor.matmul(po[:], lhsT=WoQ_sb[:, 128 * g:128 * (g + 1)],
                                     rhs=HT[g][:, 512 * b:512 * (b + 1)],
                                     start=(g == 0), stop=(g == 1))
                nc.vector.tensor_add(h1T[:, 512 * b:512 * (b + 1)], po[:],
                                     H0T[:, 512 * b:512 * (b + 1)])

            # ---- BatchNorm helper (exact, cross-core stats) ----
            def batchnorm(src, st, gst, bns, wcol, bcol, ccname):
                nc.vector.reduce_sum(out=st[:, 0:1], in_=src[:], axis=AX.X)
                nc.scalar.activation(sq[:], src[:], Act.Square,
                                     accum_out=st[:, 1:2])
                cc_in = dr.tile([128, 2], f32, name=f"{ccname}_in",
                                tag=f"{ccname}_in")
                cc_out = dr.tile([128, 2], f32, addr_space="Shared",
                                 name=f"{ccname}_out", tag=f"{ccname}_out")
                nc.sync.dma_start(cc_in[:], st[:])
                nc.gpsimd.collective_compute(
                    "AllReduce", Alu.add, replica_groups=RG,
                    ins=[cc_in[:]], outs=[cc_out[:]])
                nc.sync.dma_start(gst[:], cc_out[:])
                inv_n = 1.0 / float(NTOK)
                nc.vector.tensor_scalar_mul(bns[:, 0:1], gst[:, 0:1], inv_n)
                nc.vector.tensor_scalar_mul(bns[:, 1:2], gst[:, 1:2], inv_n)
                nc.vector.tensor_mul(bns[:, 4:5], bns[:, 0:1], bns[:, 0:1])
                nc.vector.tensor_sub(bns[:, 1:2], bns[:, 1:2], bns[:, 4:5])
                nc.scalar.activation(bns[:, 5:6], bns[:, 1:2], Act.Sqrt,
                                     bias=vecs_sb[:, 9:10])
                nc.vector.reciprocal(bns[:, 2:3], bns[:, 5:6])
                nc.vector.tensor_mul(bns[:, 2:3], bns[:, 2:3],
                                     vecs_sb[:, wcol:wcol + 1])
                nc.vector.tensor_mul(bns[:, 4:5], bns[:, 0:1], bns[:, 2:3])
                nc.vector.tensor_sub(bns[:, 3:4], vecs_sb[:, bcol:bcol + 1],
                                     bns[:, 4:5])

            # ---- BN1 ----
            batchnorm(h1T, st1, gst1, bn1s, 1, 2, "cc1")
            for c in range(4):
                nc.vector.tensor_scalar(
                    h1nT[:, 512 * c:512 * (c + 1)],
                    h1T[:, 512 * c:512 * (c + 1)],
                    bn1s[:, 2:3], bn1s[:, 3:4], op0=Alu.mult, op1=Alu.add)

            # ---- FFN (ffb2 cancels inside BN2) ----
            for qf in range(4):
                for c in range(4):
                    pf = ps.tile([128, 512], f32, tag="mm", bufs=2,
                                 name=f"pf{qf}{c}")
                    nc.tensor.matmul(pf[:],
                                     lhsT=fW1_sb[:, 128 * qf:128 * (qf + 1)],
                                     rhs=h1nT[:, 512 * c:512 * (c + 1)],
                                     start=True, stop=True)
                    nc.vector.tensor_scalar(
                        h2T[qf][:, 512 * c:512 * (c + 1)], pf[:],
                        vecs_sb[:, 3 + qf:4 + qf], 0.0,
                        op0=Alu.add, op1=Alu.max)
            for c in range(4):
                p2 = ps.tile([128, 512], f32, tag="mm", bufs=2, name=f"p2{c}")
                for qf in range(4):
                    nc.tensor.matmul(p2[:],
                                     lhsT=fW2_sb[:, 128 * qf:128 * (qf + 1)],
                                     rhs=h2T[qf][:, 512 * c:512 * (c + 1)],
                                     start=(qf == 0), stop=(qf == 3))
                nc.vector.tensor_add(yT[:, 512 * c:512 * (c + 1)], p2[:],
                                     h1nT[:, 512 * c:512 * (c + 1)])

            # ---- BN2 + output ----
            batchnorm(yT, st2, gst2, bn2s, 7, 8, "cc2")
            for c in range(4):
                nc.vector.tensor_scalar(
                    sq[:, 512 * c:512 * (c + 1)], yT[:, 512 * c:512 * (c + 1)],
                    bn2s[:, 2:3], bn2s[:, 3:4], op0=Alu.mult, op1=Alu.add)
                nc.sync.dma_start(d_yT[:, 512 * c:512 * (c + 1)],
                                  sq[:, 512 * c:512 * (c + 1)])

    nc.compile()
    return nc


def _host_prep(inputs):
    f = np.float32
    Wq, Wk, Wv, Wo = (np.asarray(inputs[k], f) for k in ("Wq", "Wk", "Wv", "Wo"))
    WqQ = np.zeros((2, E, 128), f)
    WkQ = np.zeros((2, E, 128), f)
    WoQ = np.zeros((2, 128, E), f)
    for g in range(2):
        for hh in range(4):
            h = 4 * g + hh
            WqQ[g, :, 32 * hh:32 * hh + 16] = Wq[h]
            WkQ[g, :, 32 * hh:32 * hh + 16] = Wk[h]
            WoQ[g, 32 * hh + 1:32 * hh + 17, :] = Wo[h]
    WvI = np.ascontiguousarray(np.transpose(Wv, (1, 0, 2)).reshape(E, H * KD))
    fW2 = np.ascontiguousarray(
        np.asarray(inputs["ffW2"], f).reshape(4, 128, E).transpose(1, 0, 2))
    vecs = np.zeros((128, 12), f)
    vecs[:, 0] = inputs["be1"]
    vecs[:, 1] = inputs["bn1_w"]
    vecs[:, 2] = inputs["bn1_b"]
    vecs[:, 3:7] = np.asarray(inputs["ffb1"], f).reshape(4, 128).T
    vecs[:, 7] = inputs["bn2_w"]
    vecs[:, 8] = inputs["bn2_b"]
    vecs[:, 9] = EPS
    return {
        "We1": np.ascontiguousarray(np.asarray(inputs["We1"], f)),
        "WqQ": np.ascontiguousarray(np.concatenate([WqQ[0], WqQ[1]], axis=1)),
        "WkQ": np.ascontiguousarray(np.concatenate([WkQ[0], WkQ[1]], axis=1)),
        "WvI": WvI,
        "WoQ": np.ascontiguousarray(np.concatenate([WoQ[0], WoQ[1]], axis=1)),
        "fW1": np.ascontiguousarray(np.asarray(inputs["ffW1"], f)),
        "fW2": np.ascontiguousarray(fW2.reshape(128, 512)), "vecs": vecs,
    }


def _get_runner():
    """Build the sharded jitted executable once and cache it."""
    if "runner" in _CACHE:
        return _CACHE["runner"]
    import jax
    import concourse.mybir as mybir
    from jax.sharding import Mesh, PartitionSpec
    from jax.experimental.shard_map import shard_map
    from concourse.bass2jax import (_bass_exec_p, install_neuronx_cc_hook,
                                    partition_id_tensor)

    if "nc" not in _CACHE:
        _CACHE["nc"] = _build_nc()
    nc = _CACHE["nc"]
    install_neuronx_cc_hook()
    assert nc.dbg_addr is None

    partition_name = (nc.partition_id_tensor.name
                      if nc.partition_id_tensor else None)
    in_names, out_names, out_avals, zero_outs = [], [], [], []
    for alloc in nc.m.functions[0].allocations:
        if not isinstance(alloc, mybir.MemoryLocationSet):
            continue
        name = alloc.memorylocations[0].name
        if alloc.kind == "ExternalInput":
            if name != partition_name:
                in_names.append(name)
        elif alloc.kind == "ExternalOutput":
            shape = tuple(alloc.tensor_shape)
            dtype = mybir.dt.np(alloc.dtype)
            out_names.append(name)
            out_avals.append(jax.core.ShapedArray(shape, dtype))
            zero_outs.append(np.zeros(shape, dtype))
    n_params = len(in_names)
    n_outs = len(out_avals)
    all_in_names = list(in_names) + list(out_names)
    if partition_name is not None:
        all_in_names.append(partition_name)
    donate = tuple(range(n_params, n_params + n_outs))

    def _body(*args):
        operands = list(args)
        if partition_name is not None:
            operands.append(partition_id_tensor())
        outs = _bass_exec_p.bind(
            *operands,
            out_avals=tuple(out_avals),
            in_names=tuple(all_in_names),
            out_names=tuple(out_names),
            lowering_input_output_aliases=(),
            sim_require_finite=True,
            sim_require_nnan=True,
            nc=nc,
        )
        return tuple(outs)

    devices = jax.devices()[:NCORES]
    mesh = Mesh(np.asarray(devices), ("core",))
    in_specs = (PartitionSpec("core"),) * (n_params + n_outs)
    out_specs = (PartitionSpec("core"),) * len(out_names)
    sharded = jax.jit(
        shard_map(_body, mesh=mesh, in_specs=in_specs, out_specs=out_specs,
                  check_rep=False),
        donate_argnums=donate, keep_unused=True)

    def run(in_maps):
        per_core = [[np.asarray(m[name]) for name in in_names]
                    for m in in_maps]
        concat_in = [np.concatenate([per_core[c][i] for c in range(NCORES)],
                                    axis=0) for i in range(n_params)]
        concat_zeros = [np.zeros((NCORES * z.shape[0], *z.shape[1:]), z.dtype)
                        for z in zero_outs]
        out_arrs = sharded(*concat_in, *concat_zeros)
        out_arrs = [np.asarray(a) for a in out_arrs]
        return [{name: out_arrs[i].reshape(NCORES, *out_avals[i].shape)[c]
                 for i, name in enumerate(out_names)}
                for c in range(NCORES)]

    _CACHE["runner"] = run
    return run


def _make_in_maps(inputs):
    shared = _host_prep(inputs)
    x1 = np.asarray(inputs["x1"], np.float32)
    in_maps = []
    for cidx in range(NCORES):
        m = dict(shared)
        xl = x1[BPC * cidx:BPC * (cidx + 1)].reshape(T, D_IN)
        m["xT"] = np.ascontiguousarray(xl.T)
        in_maps.append(m)
    return in_maps


def kernel(**inputs):
    run = _get_runner()
    results = run(_make_in_maps(inputs))
    outs = []
    for cidx in range(NCORES):
        yTo = results[cidx]["yT"]          # [E, T]
        outs.append(np.ascontiguousarray(yTo.T).reshape(BPC, N, E))
    return np.concatenate(outs, 0).astype(np.float32)


# revision 7
# speedup vs baseline: 2781.6616x; 1470.9085x over previous
"""Trainium2 Bass kernel for nn_MHABlock (dense transformer block).

Sharding: data-parallel over batch — 8 cores x 4 batches (2048 tokens/core).
BatchNorm stats are exact via two tiny cross-core AllReduces ([128,2] each).

On-device layout is E-major ("hT" = [E(128 partitions), tokens(free)]) so
BatchNorm / bias / affine ops are per-partition scalars. Attention uses the
"scoresT" formulation (scores transposed: [k_tok, q_tok]): softmax sums
arrive free from an all-ones column prepended to V (row 32*hh of the attnV
PSUM = sum_k exp), no transpose of the attention matrix is needed, and the
exp output feeds attnV directly as the streaming operand. Per-head softmax
denominators land on partitions {0,32,64,96}; reciprocal_approx_fast over
the whole bank + K=1 outer-product matmuls with a ones vector broadcast
1/sum across each head's 32-partition group, so normalization is a single
[128,512] tensor_tensor multiply. Zero rows in the out-projection weights
kill all pad rows. q/k/exp/V run in bf16 (PE fp32 is half-rate).
"""

import numpy as np

B, N, D_IN, E, H, KD, FF = 32, 512, 2, 128, 8, 16, 512
NCORES = 8
BPC = B // NCORES          # batches per core
T = BPC * N                # 2048 local tokens
NTOK = B * N               # global token count for BN
NORM = 1.0 / np.sqrt(16.0)
EPS = 1e-5

_CACHE = {}
LAST_RESULT = None


def _build_nc():
    import concourse.bass as bass  # noqa: F401
    import concourse.mybir as mybir
    import concourse.tile as tile
    from concourse import bacc

    f32 = mybir.dt.float32
    bf16 = mybir.dt.bfloat16
    Act = mybir.ActivationFunctionType
    Alu = mybir.AluOpType
    AX = mybir.AxisListType

    nc = bacc.Bacc("TRN2", target_bir_lowering=False, debug=False,
                   enable_asserts=False, num_devices=NCORES)

    # ---- DRAM I/O ----
    d_xT = nc.dram_tensor("xT", [D_IN, T], f32, kind="ExternalInput").ap()
    d_We1 = nc.dram_tensor("We1", [D_IN, E], f32, kind="ExternalInput").ap()
    d_WqQ = nc.dram_tensor("WqQ", [E, 256], bf16, kind="ExternalInput").ap()
    d_WkQ = nc.dram_tensor("WkQ", [E, 256], bf16, kind="ExternalInput").ap()
    d_WvI = nc.dram_tensor("WvI", [E, 128], bf16, kind="ExternalInput").ap()
    d_WoQ = nc.dram_tensor("WoQ", [128, 256], bf16, kind="ExternalInput").ap()
    d_fW1 = nc.dram_tensor("fW1", [E, FF], bf16, kind="ExternalInput").ap()
    d_fW2 = nc.dram_tensor("fW2", [128, 512], bf16, kind="ExternalInput").ap()
    d_vecs = nc.dram_tensor("vecs", [128, 12], f32, kind="ExternalInput").ap()
    d_yT = nc.dram_tensor("yT", [E, T], f32, kind="ExternalOutput").ap()

    RG = [list(range(NCORES))]

    with tile.TileContext(nc) as tc:
        with tc.sbuf_pool(name="sb", bufs=1) as sb, \
             tc.psum_pool(name="ps", bufs=1) as ps, \
             tc.tile_pool(name="dr", bufs=1, space="DRAM") as dr:

            def P(shape, dt, name):  # persistent tile
                return sb.tile(shape, dt, name=name, tag=name, bufs=1)

            xT = P([D_IN, T], f32, "xT_sb")
            We1_sb = P([D_IN, E], f32, "We1_sb")
            WqQ_sb = P([128, 256], bf16, "WqQ_sb")
            WkQ_sb = P([128, 256], bf16, "WkQ_sb")
            WvI_sb = P([128, 128], bf16, "WvI_sb")
            WoQ_sb = P([128, 256], bf16, "WoQ_sb")
            fW1_sb = P([128, FF], bf16, "fW1_sb")
            fW2_sb = P([128, 512], bf16, "fW2_sb")
            vecs_sb = P([128, 12], f32, "vecs_sb")
            ones_sb = P([128, 32], f32, "ones_sb")

            H0T = P([128, T], f32, "H0T")
            H0b = P([128, T], bf16, "H0b")
            qT = [P([128, T], bf16, f"qT{g}") for g in range(2)]
            kT = [P([128, T], bf16, f"kT{g}") for g in range(2)]
            V_aug = P([128, 16 * 256], bf16, "V_aug")
            HT = [P([128, T], bf16, f"HT{g}") for g in range(2)]
            h1T = P([128, T], f32, "h1T")
            h1nT = P([128, T], f32, "h1nT")
            h1nb = P([128, T], bf16, "h1nb")
            h2T = [P([128, T], bf16, f"h2T{qf}") for qf in range(4)]
            yT = P([128, T], f32, "yT_sb")
            sq = P([128, T], f32, "sq")
            st1 = P([128, 2], f32, "st1")
            st2 = P([128, 2], f32, "st2")
            gst1 = P([128, 2], f32, "gst1")
            gst2 = P([128, 2], f32, "gst2")
            bn1s = P([128, 6], f32, "bn1s")
            bn2s = P([128, 6], f32, "bn2s")

            # ---- load inputs ----
            nc.sync.dma_start(xT[:], d_xT)
            nc.sync.dma_start(We1_sb[:], d_We1)
            nc.sync.dma_start(WqQ_sb[:], d_WqQ)
            nc.sync.dma_start(WkQ_sb[:], d_WkQ)
            nc.sync.dma_start(WvI_sb[:], d_WvI)
            nc.sync.dma_start(WoQ_sb[:], d_WoQ)
            nc.sync.dma_start(fW1_sb[:], d_fW1)
            nc.sync.dma_start(fW2_sb[:], d_fW2)
            nc.sync.dma_start(vecs_sb[:], d_vecs)
            nc.vector.memset(ones_sb[:], 1.0)
            nc.vector.memset(V_aug[:], 0.0)
            va_ones = V_aug.rearrange("p (t h w) -> p (t h) w", t=16, h=8)[:, :, 0:1]
            nc.gpsimd.memset(va_ones, 1.0)

            # ---- Phase A: embedding h0 = x @ We1 + be1 (E-major) ----
            for c in range(4):
                pm = ps.tile([128, 512], f32, tag="mm", bufs=2, name=f"pm_e{c}")
                nc.tensor.matmul(pm[:], lhsT=We1_sb[:],
                                 rhs=xT[:, 512 * c:512 * (c + 1)],
                                 start=True, stop=True)
                nc.vector.tensor_scalar_add(H0T[:, 512 * c:512 * (c + 1)],
                                            pm[:], vecs_sb[:, 0:1])

            nc.vector.tensor_copy(H0b[:], H0T[:])

            # ---- Phase B: q/k projections (quad-padded, bf16 out) ----
            for g in range(2):
                for c in range(4):
                    pq = ps.tile([128, 512], f32, tag="mm", bufs=2,
                                 name=f"pq{g}{c}")
                    nc.tensor.matmul(pq[:], lhsT=WqQ_sb[:, 128 * g:128 * (g + 1)],
                                     rhs=H0b[:, 512 * c:512 * (c + 1)],
                                     start=True, stop=True)
                    nc.vector.tensor_copy(qT[g][:, 512 * c:512 * (c + 1)], pq[:])
                    pk = ps.tile([128, 512], f32, tag="mm", bufs=2,
                                 name=f"pk{g}{c}")
                    nc.tensor.matmul(pk[:], lhsT=WkQ_sb[:, 128 * g:128 * (g + 1)],
                                     rhs=H0b[:, 512 * c:512 * (c + 1)],
                                     start=True, stop=True)
                    nc.vector.tensor_copy(kT[g][:, 512 * c:512 * (c + 1)], pk[:])

            # ---- v projection into V_aug (token-major, 32-blocks +ones) ----
            for t in range(16):
                pv = ps.tile([128, 128], f32, tag="mm", bufs=2, name=f"pv{t}")
                nc.tensor.matmul(pv[:], lhsT=H0b[:, 128 * t:128 * (t + 1)],
                                 rhs=WvI_sb[:], start=True, stop=True)
                dst = V_aug[:, 256 * t:256 * (t + 1)]
                dst = dst.rearrange("p (h w) -> p h w", h=8)[:, :, 1:17]
                src = pv.rearrange("p (h w) -> p h w", h=8)
                nc.vector.tensor_copy(dst, src)

            # ---- Phase C: attention ----
            for b in range(4):
                for g in range(2):
                    av = ps.tile([128, 512], f32, tag="av", bufs=2,
                                 name=f"av{b}{g}")
                    for c in range(4):
                        for hp in range(2):
                            scp = ps.tile([128, 1024], f32, tag="sc", bufs=2,
                                          name=f"scp{b}{g}{c}{hp}")
                            for j in range(2):
                                hh = 2 * hp + j
                                nc.tensor.matmul(
                                    scp[:, 512 * j:512 * (j + 1)],
                                    lhsT=kT[g][32 * hh:32 * (hh + 1),
                                               512 * b + 128 * c:
                                               512 * b + 128 * (c + 1)],
                                    rhs=qT[g][32 * hh:32 * (hh + 1),
                                              512 * b:512 * (b + 1)],
                                    start=True, stop=True,
                                    tile_position=(32 * hh, 0))
                            ex = sb.tile([128, 1024], bf16, tag="ex", bufs=3,
                                         name=f"ex{b}{g}{c}{hp}")
                            nc.scalar.activation(ex[:], scp[:], Act.Exp,
                                                 scale=float(NORM))
                            for j in range(2):
                                hh = 2 * hp + j
                                h = 4 * g + hh
                                tci = 4 * b + c
                                nc.tensor.matmul(
                                    av[32 * hh:32 * (hh + 1), :],
                                    lhsT=V_aug[:, 256 * tci + 32 * h:
                                               256 * tci + 32 * h + 32],
                                    rhs=ex[:, 512 * j:512 * (j + 1)],
                                    start=(c == 0), stop=(c == 3),
                                    tile_position=(0, 32 * hh))
                    # normalize: raw * (1/sums), broadcast via K=1 matmul
                    raw = sb.tile([128, 512], f32, tag="raw", bufs=2,
                                  name=f"raw{b}{g}")
                    nc.vector.tensor_copy(raw[:], av[:])
                    rec = sb.tile([128, 512], f32, tag="rec", bufs=2,
                                  name=f"rec{b}{g}")
                    nc.vector.reciprocal_approx_fast(rec[:], av[:])
                    Rp = ps.tile([128, 512], f32, tag="mm", bufs=2,
                                 name=f"Rp{b}{g}")
                    for hh in range(4):
                        nc.tensor.matmul(
                            Rp[32 * hh:32 * (hh + 1), :],
                            lhsT=ones_sb[32 * hh:32 * hh + 1, :],
                            rhs=rec[32 * hh:32 * hh + 1, :],
                            start=True, stop=True,
                            tile_position=(32 * hh, 32 * hh))
                    nc.vector.tensor_mul(HT[g][:, 512 * b:512 * (b + 1)],
                                         raw[:], Rp[:])

                # out-projection + skip (both quads accumulate in PSUM)
                po = ps.tile([128, 512], f32, tag="mm", bufs=2, name=f"po{b}")
                for g in range(2):
                    nc.tensor.matmul(po[:], lhsT=WoQ_sb[:, 128 * g:128 * (g + 1)],
                                     rhs=HT[g][:, 512 * b:512 * (b + 1)],
                                     start=(g == 0), stop=(g == 1))
                nc.vector.tensor_add(h1T[:, 512 * b:512 * (b + 1)], po[:],
                                     H0T[:, 512 * b:512 * (b + 1)])

            # ---- BatchNorm helper (exact, cross-core stats) ----
            def batchnorm(src, st, gst, bns, wcol, bcol, ccname):
                nc.vector.reduce_sum(out=st[:, 0:1], in_=src[:], axis=AX.X)
                nc.scalar.activation(sq[:], src[:], Act.Square,
                                     accum_out=st[:, 1:2])
                cc_in = dr.tile([128, 2], f32, name=f"{ccname}_in",
                                tag=f"{ccname}_in")
                cc_out = dr.tile([128, 2], f32, addr_space="Shared",
                                 name=f"{ccname}_out", tag=f"{ccname}_out")
                nc.sync.dma_start(cc_in[:], st[:])
                nc.gpsimd.collective_compute(
                    "AllReduce", Alu.add, replica_groups=RG,
                    ins=[cc_in[:]], outs=[cc_out[:]])
                nc.sync.dma_start(gst[:], cc_out[:])
                inv_n = 1.0 / float(NTOK)
                nc.vector.tensor_scalar_mul(bns[:, 0:1], gst[:, 0:1], inv_n)
                nc.vector.tensor_scalar_mul(bns[:, 1:2], gst[:, 1:2], inv_n)
                nc.vector.tensor_mul(bns[:, 4:5], bns[:, 0:1], bns[:, 0:1])
                nc.vector.tensor_sub(bns[:, 1:2], bns[:, 1:2], bns[:, 4:5])
                nc.scalar.activation(bns[:, 5:6], bns[:, 1:2], Act.Sqrt,
                                     bias=vecs_sb[:, 9:10])
                nc.vector.reciprocal(bns[:, 2:3], bns[:, 5:6])
                nc.vector.tensor_mul(bns[:, 2:3], bns[:, 2:3],
                                     vecs_sb[:, wcol:wcol + 1])
                nc.vector.tensor_mul(bns[:, 4:5], bns[:, 0:1], bns[:, 2:3])
                nc.vector.tensor_sub(bns[:, 3:4], vecs_sb[:, bcol:bcol + 1],
                                     bns[:, 4:5])

            # ---- BN1 ----
            batchnorm(h1T, st1, gst1, bn1s, 1, 2, "cc1")
            for c in range(4):
                nc.vector.tensor_scalar(
                    h1nT[:, 512 * c:512 * (c + 1)],
                    h1T[:, 512 * c:512 * (c + 1)],
                    bn1s[:, 2:3], bn1s[:, 3:4], op0=Alu.mult, op1=Alu.add)
                nc.vector.tensor_copy(h1nb[:, 512 * c:512 * (c + 1)],
                                      h1nT[:, 512 * c:512 * (c + 1)])

            # ---- FFN (ffb2 cancels inside BN2) ----
            for qf in range(4):
                for c in range(4):
                    pf = ps.tile([128, 512], f32, tag="mm", bufs=2,
                                 name=f"pf{qf}{c}")
                    nc.tensor.matmul(pf[:],
                                     lhsT=fW1_sb[:, 128 * qf:128 * (qf + 1)],
                                     rhs=h1nb[:, 512 * c:512 * (c + 1)],
                                     start=True, stop=True)
                    nc.vector.tensor_scalar(
                        h2T[qf][:, 512 * c:512 * (c + 1)], pf[:],
                        vecs_sb[:, 3 + qf:4 + qf], 0.0,
                        op0=Alu.add, op1=Alu.max)
            for c in range(4):
                p2 = ps.tile([128, 512], f32, tag="mm", bufs=2, name=f"p2{c}")
                for qf in range(4):
                    nc.tensor.matmul(p2[:],
                                     lhsT=fW2_sb[:, 128 * qf:128 * (qf + 1)],
                                     rhs=h2T[qf][:, 512 * c:512 * (c + 1)],
                                     start=(qf == 0), stop=(qf == 3))
                nc.vector.tensor_add(yT[:, 512 * c:512 * (c + 1)], p2[:],
                                     h1nT[:, 512 * c:512 * (c + 1)])

            # ---- BN2 + output ----
            batchnorm(yT, st2, gst2, bn2s, 7, 8, "cc2")
            for c in range(4):
                nc.vector.tensor_scalar(
                    sq[:, 512 * c:512 * (c + 1)], yT[:, 512 * c:512 * (c + 1)],
                    bn2s[:, 2:3], bn2s[:, 3:4], op0=Alu.mult, op1=Alu.add)
                nc.sync.dma_start(d_yT[:, 512 * c:512 * (c + 1)],
                                  sq[:, 512 * c:512 * (c + 1)])

    nc.compile()
    return nc


def _host_prep(inputs):
    f = np.float32
    Wq, Wk, Wv, Wo = (np.asarray(inputs[k], f) for k in ("Wq", "Wk", "Wv", "Wo"))
    WqQ = np.zeros((2, E, 128), f)
    WkQ = np.zeros((2, E, 128), f)
    WoQ = np.zeros((2, 128, E), f)
    for g in range(2):
        for hh in range(4):
            h = 4 * g + hh
            WqQ[g, :, 32 * hh:32 * hh + 16] = Wq[h]
            WkQ[g, :, 32 * hh:32 * hh + 16] = Wk[h]
            WoQ[g, 32 * hh + 1:32 * hh + 17, :] = Wo[h]
    WvI = np.ascontiguousarray(np.transpose(Wv, (1, 0, 2)).reshape(E, H * KD))
    fW2 = np.ascontiguousarray(
        np.asarray(inputs["ffW2"], f).reshape(4, 128, E).transpose(1, 0, 2))
    vecs = np.zeros((128, 12), f)
    vecs[:, 0] = inputs["be1"]
    vecs[:, 1] = inputs["bn1_w"]
    vecs[:, 2] = inputs["bn1_b"]
    vecs[:, 3:7] = np.asarray(inputs["ffb1"], f).reshape(4, 128).T
    vecs[:, 7] = inputs["bn2_w"]
    vecs[:, 8] = inputs["bn2_b"]
    vecs[:, 9] = EPS
    import ml_dtypes
    bf = ml_dtypes.bfloat16
    return {
        "We1": np.ascontiguousarray(np.asarray(inputs["We1"], f)),
        "WqQ": np.ascontiguousarray(np.concatenate([WqQ[0], WqQ[1]], axis=1)).astype(bf),
        "WkQ": np.ascontiguousarray(np.concatenate([WkQ[0], WkQ[1]], axis=1)).astype(bf),
        "WvI": WvI.astype(bf),
        "WoQ": np.ascontiguousarray(np.concatenate([WoQ[0], WoQ[1]], axis=1)).astype(bf),
        "fW1": np.ascontiguousarray(np.asarray(inputs["ffW1"], f)).astype(bf),
        "fW2": np.ascontiguousarray(fW2.reshape(128, 512)).astype(bf), "vecs": vecs,
    }


def _get_runner():
    """Build the sharded jitted executable once and cache it."""
    if "runner" in _CACHE:
        return _CACHE["runner"]
    import jax
    import concourse.mybir as mybir
    from jax.sharding import Mesh, PartitionSpec
    from jax.experimental.shard_map import shard_map
    from concourse.bass2jax import (_bass_exec_p, install_neuronx_cc_hook,
                                    partition_id_tensor)

    if "nc" not in _CACHE:
        _CACHE["nc"] = _build_nc()
    nc = _CACHE["nc"]
    install_neuronx_cc_hook()
    assert nc.dbg_addr is None

    partition_name = (nc.partition_id_tensor.name
                      if nc.partition_id_tensor else None)
    in_names, out_names, out_avals, zero_outs = [], [], [], []
    for alloc in nc.m.functions[0].allocations:
        if not isinstance(alloc, mybir.MemoryLocationSet):
            continue
        name = alloc.memorylocations[0].name
        if alloc.kind == "ExternalInput":
            if name != partition_name:
                in_names.append(name)
        elif alloc.kind == "ExternalOutput":
            shape = tuple(alloc.tensor_shape)
            dtype = mybir.dt.np(alloc.dtype)
            out_names.append(name)
            out_avals.append(jax.core.ShapedArray(shape, dtype))
            zero_outs.append(np.zeros(shape, dtype))
    n_params = len(in_names)
    n_outs = len(out_avals)
    all_in_names = list(in_names) + list(out_names)
    if partition_name is not None:
        all_in_names.append(partition_name)
    donate = tuple(range(n_params, n_params + n_outs))

    def _body(*args):
        operands = list(args)
        if partition_name is not None:
            operands.append(partition_id_tensor())
        outs = _bass_exec_p.bind(
            *operands,
            out_avals=tuple(out_avals),
            in_names=tuple(all_in_names),
            out_names=tuple(out_names),
            lowering_input_output_aliases=(),
            sim_require_finite=True,
            sim_require_nnan=True,
            nc=nc,
        )
        return tuple(outs)

    devices = jax.devices()[:NCORES]
    mesh = Mesh(np.asarray(devices), ("core",))
    in_specs = (PartitionSpec("core"),) * (n_params + n_outs)
    out_specs = (PartitionSpec("core"),) * len(out_names)
    sharded = jax.jit(
        shard_map(_body, mesh=mesh, in_specs=in_specs, out_specs=out_specs,
                  check_rep=False),
        donate_argnums=donate, keep_unused=True)

    def run(in_maps):
        per_core = [[np.asarray(m[name]) for name in in_names]
                    for m in in_maps]
        concat_in = [np.concatenate([per_core[c][i] for c in range(NCORES)],
                                    axis=0) for i in range(n_params)]
        concat_zeros = [np.zeros((NCORES * z.shape[0], *z.shape[1:]), z.dtype)
                        for z in zero_outs]
        out_arrs = sharded(*concat_in, *concat_zeros)
        out_arrs = [np.asarray(a) for a in out_arrs]
        return [{name: out_arrs[i].reshape(NCORES, *out_avals[i].shape)[c]
                 for i, name in enumerate(out_names)}
                for c in range(NCORES)]

    _CACHE["runner"] = run
    return run


def _make_in_maps(inputs):
    shared = _host_prep(inputs)
    x1 = np.asarray(inputs["x1"], np.float32)
    in_maps = []
    for cidx in range(NCORES):
        m = dict(shared)
        xl = x1[BPC * cidx:BPC * (cidx + 1)].reshape(T, D_IN)
        m["xT"] = np.ascontiguousarray(xl.T)
        in_maps.append(m)
    return in_maps


def kernel(**inputs):
    run = _get_runner()
    results = run(_make_in_maps(inputs))
    outs = []
    for cidx in range(NCORES):
        yTo = results[cidx]["yT"]          # [E, T]
        outs.append(np.ascontiguousarray(yTo.T).reshape(BPC, N, E))
    return np.concatenate(outs, 0).astype(np.float32)
